# revision 26
# baseline (speedup 1.0000x reference)
"""MoE FFN (8 experts, top-2) on 8 TRN2 NeuronCores, expert-parallel.

Strategy:
  - Host: router (fp64 logits -> softmax -> top-2 -> renormalized combine
    weights), gather each expert's assigned tokens, pad to a common
    capacity C (SPMD: one program, per-core inputs).
  - Core e: full SwiGLU FFN for expert e over its C tokens in bf16
    (full PE rate, ~40 MB HBM traffic/rep), combine-weight scaling on
    device; outputs [C, 1024].
  - Host: scatter-add per-expert outputs back into [B, S, D].

Layouts (host-prepared, DMA-friendly):
  xT   [8, 128, C]       x[idx].T split along d into 8 k-tiles (bf16)
  gw/uw[32, 128, 8, 128] gate/up ^T tiled: [h_tile][d_sub][k][h] (bf16)
  dw   [32, 128, 1024]   down^T tiled:     [h_tile][h_sub][dout] (bf16)
  cwT  [128, ceil(C/128)] combine weights, partition-major (f32)

Device schedule (per rep):
  Phase 1: for each of 32 h-tiles, gate+up matmuls over ~416-token PSUM
    groups (8 accumulating k-matmuls each, same PSUM bank per chain),
    silu on Act engine, h=sg*u on DVE into a resident bf16 hbuf
    [128, 32, C].
  Phase 2: down-proj with TOKENS as the moving dim (512-wide): per
    dout-tile (8 of 128), a dw slab [128, 32, 128]; per token group a
    single-bank 32-matmul accumulation chain (consecutive matmuls share
    the PSUM bank so the PE hides stationary loads); elementwise
    combine-weight scale on DVE (cw replicated across partitions), DMA
    out in [dout, token] layout; host transposes (free).

C is a multiple of 32 (tail token group of C%512 keeps capacity tight).
"""
import sys, os
for p in ("/opt/trn_rl_repo", os.path.join(os.path.dirname(os.path.abspath(__file__)))):
    if p not in sys.path:
        sys.path.insert(0, p)
import numpy as np
import ml_dtypes

D_MODEL = 1024
D_INNER = 4096
N_EXPERTS = 8
TOP_K = 2
H_TILES = D_INNER // 128  # 32
K_TILES = D_MODEL // 128  # 8


def _capacity(max_n: int) -> int:
    return max(256, ((max_n + 31) // 32) * 32)


def _token_groups(C: int):
    """Near-equal gate/up token groups, each a multiple of 32, <= 512."""
    n = (C + 511) // 512
    t32 = C // 32
    base, rem = divmod(t32, n)
    out, g0 = [], 0
    for i in range(n):
        gsz = (base + (1 if i < rem else 0)) * 32
        out.append((g0, gsz))
        g0 += gsz
    return out


def _build_nc(C: int, reps: int = 1, unroll: int = 8):
    import concourse.bass as bass
    import concourse.mybir as mybir
    import concourse.tile as tile
    from concourse import bacc
    from contextlib import nullcontext

    f32 = mybir.dt.float32
    bf16 = mybir.dt.bfloat16
    Silu = mybir.ActivationFunctionType.Silu

    assert C % 32 == 0
    groups = _token_groups(C)

    nc = bacc.Bacc(None, target_bir_lowering=False)
    xT_d = nc.dram_tensor("xT", [K_TILES, 128, C], bf16, kind="ExternalInput")
    # gate+up packed: [h_tile][d_sub][2(g,u)][k][h]
    guw_d = nc.dram_tensor("guw", [H_TILES, 128, 2, K_TILES, 128], bf16,
                           kind="ExternalInput")
    # dw packed per dout-tile: [dout_tile][h_sub][h_tile][dout_sub]
    dw_d = nc.dram_tensor("dw", [D_MODEL // 128, 128, H_TILES, 128], bf16,
                          kind="ExternalInput")
    cw_d = nc.dram_tensor("cwR", [128, C], f32, kind="ExternalInput")
    y_d = nc.dram_tensor("y", [D_MODEL, C], f32, kind="ExternalOutput")

    with tile.TileContext(nc) as tc:
        with (
            tc.tile_pool(name="xt", bufs=1) as xt_pool,
            tc.tile_pool(name="wgt", bufs=3) as wgt_pool,
            tc.tile_pool(name="dwp", bufs=2) as dw_pool,
            tc.tile_pool(name="hb", bufs=1) as hb_pool,
            tc.tile_pool(name="sg", bufs=2) as sg_pool,
            tc.tile_pool(name="yo", bufs=3) as y_pool,
            tc.tile_pool(name="cw", bufs=1) as cw_pool,
            tc.tile_pool(name="ps1", bufs=3, space="PSUM") as ps1,
            tc.tile_pool(name="ps2", bufs=5, space="PSUM") as ps2,
        ):
            cw_sb = cw_pool.tile([128, C], f32)
            nc.sync.dma_start(cw_sb[:], cw_d[:])

            def emit_rep():
                # x streamed per token group into separate tiles so the first
                # gate matmul only waits on its own group's DMA.
                xts = []
                for gi, (g0, gsz) in enumerate(groups):
                    xg = xt_pool.tile([128, K_TILES, gsz], bf16, tag=f"xt{gi}",
                                      name="xg")
                    # DRAM [k, d, t] -> SBUF [d, k, t] (transpose on the DRAM
                    # side so the SBUF AP stays partition-major)
                    nc.sync.dma_start(xg[:], xT_d[:, :, g0:g0 + gsz].transpose([1, 0, 2]))
                    xts.append(xg)
                hbuf = hb_pool.tile([128, H_TILES, C], bf16, tag="hbuf")

                # ---- phase 1: gate/up + SwiGLU, weights streamed once
                for hi in range(H_TILES):
                    guw = wgt_pool.tile([128, 2, K_TILES, 128], bf16, tag="w")
                    nc.sync.dma_start(guw[:], guw_d[hi])
                    for gi, (g0, gsz) in enumerate(groups):
                        hs = slice(g0, g0 + gsz)
                        xg = xts[gi]
                        pg = ps1.tile([128, gsz], f32, tag="p1", name="pg",
                                      padded_shape=[128, 512])
                        for k in range(K_TILES):
                            nc.tensor.matmul(pg[:], guw[:, 0, k, :], xg[:, k, :],
                                             start=(k == 0), stop=(k == K_TILES - 1))
                        pu = ps1.tile([128, gsz], f32, tag="p1", name="pu",
                                      padded_shape=[128, 512])
                        for k in range(K_TILES):
                            nc.tensor.matmul(pu[:], guw[:, 1, k, :], xg[:, k, :],
                                             start=(k == 0), stop=(k == K_TILES - 1))
                        sg = sg_pool.tile([128, gsz], bf16, tag="sg", name="sg",
                                          padded_shape=[128, 512])
                        nc.scalar.activation(sg[:], pg[:], Silu)
                        nc.vector.tensor_mul(hbuf[:, hi, hs], sg[:], pu[:])

                # ---- phase 2: down-projection, tokens moving (512-wide),
                # same-bank 32-matmul accumulation chains per token group
                for dt in range(D_MODEL // 128):
                    dwt = dw_pool.tile([128, H_TILES, 128], bf16, tag="dw", name="dwt")
                    nc.sync.dma_start(dwt[:], dw_d[dt])
                    for (g0, gsz) in groups:
                        gs = slice(g0, g0 + gsz)
                        yp = ps2.tile([128, gsz], f32, tag="p2", name="yp",
                                      padded_shape=[128, 512])
                        for hi in range(H_TILES):
                            nc.tensor.matmul(
                                yp[:], dwt[:, hi, :], hbuf[:, hi, gs],
                                start=(hi == 0), stop=(hi == H_TILES - 1))
                        yt = y_pool.tile([128, gsz], f32, tag="yt", name="yt",
                                         padded_shape=[128, 512])
                        nc.vector.tensor_mul(yt[:], yp[:], cw_sb[:, gs])
                        nc.sync.dma_start(y_d[dt * 128:(dt + 1) * 128, gs], yt[:])

            # Unroll the rep loop (per-For_i-iteration overhead is ~30 ns/rep
            # x measured ~30 us at unroll=1); tail reps emitted inline.
            n_loop, tail = divmod(reps, unroll)
            if n_loop > 1:
                with tc.For_i(0, n_loop, 1):
                    for _u in range(unroll):
                        emit_rep()
                for _u in range(tail):
                    emit_rep()
            else:
                for _u in range(max(1, reps)):
                    emit_rep()
    nc.finalize()
    return nc


_NC_CACHE: dict = {}


def _get_nc(C: int):
    if C not in _NC_CACHE:
        _NC_CACHE[C] = _build_nc(C)
    return _NC_CACHE[C]


def _route(x2d: np.ndarray, router_w: np.ndarray, router_b: np.ndarray):
    """fp64 router: returns (idx_per_expert, cw_per_expert) lists."""
    logits = x2d.astype(np.float64) @ router_w.astype(np.float64).T + router_b.astype(np.float64)
    m = logits.max(axis=-1, keepdims=True)
    p = np.exp(logits - m)
    p /= p.sum(axis=-1, keepdims=True)
    # top-2 (jax.lax.top_k picks largest; softmax is monotonic in logits)
    i1 = np.argmax(p, axis=-1)
    p_masked = p.copy()
    p_masked[np.arange(p.shape[0]), i1] = -1.0
    i2 = np.argmax(p_masked, axis=-1)
    p1 = p[np.arange(p.shape[0]), i1]
    p2 = p[np.arange(p.shape[0]), i2]
    denom = p1 + p2
    w1 = p1 / denom
    w2 = p2 / denom
    idxs, cws = [], []
    for e in range(N_EXPERTS):
        sel1 = np.nonzero(i1 == e)[0]
        sel2 = np.nonzero(i2 == e)[0]
        idx = np.concatenate([sel1, sel2])
        cw = np.concatenate([w1[sel1], w2[sel2]])
        idxs.append(idx)
        cws.append(cw.astype(np.float32))
    return idxs, cws


def _prep_core_inputs(x2d, idxs, cws, gate_w, up_w, down_w, C):
    bf16 = ml_dtypes.bfloat16
    in_maps = []
    for e in range(N_EXPERTS):
        idx = idxs[e]
        n = len(idx)
        xe = np.zeros((C, D_MODEL), np.float32)
        xe[:n] = x2d[idx]
        xT = np.ascontiguousarray(xe.T).astype(bf16).reshape(K_TILES, 128, C)
        gw = gate_w[e].T.reshape(K_TILES, 128, H_TILES, 128).transpose(2, 1, 0, 3)
        uw = up_w[e].T.reshape(K_TILES, 128, H_TILES, 128).transpose(2, 1, 0, 3)
        # [h_tile][d_sub][2(g,u)][k][h]
        guw = np.ascontiguousarray(
            np.stack([gw, uw], axis=2)
        ).astype(bf16)
        # [dout_tile][h_sub][h_tile][dout_sub]
        dw = np.ascontiguousarray(
            down_w[e].T.reshape(H_TILES, 128, D_MODEL // 128, 128).transpose(2, 1, 0, 3)
        ).astype(bf16)
        cw = np.zeros((C,), np.float32)
        cw[:n] = cws[e]
        cwR = np.ascontiguousarray(np.broadcast_to(cw[None, :], (128, C)))
        in_maps.append({"xT": xT, "guw": guw, "dw": dw, "cwR": cwR})
    return in_maps


def kernel(x, router_w, router_b, gate_w, up_w, down_w):
    from concourse.bass_utils import run_bass_kernel_spmd

    x = np.asarray(x, dtype=np.float32)
    router_w = np.asarray(router_w, dtype=np.float32)
    router_b = np.asarray(router_b, dtype=np.float32)
    gate_w = np.asarray(gate_w, dtype=np.float32)
    up_w = np.asarray(up_w, dtype=np.float32)
    down_w = np.asarray(down_w, dtype=np.float32)

    B, S, D = x.shape
    x2d = x.reshape(B * S, D)
    idxs, cws = _route(x2d, router_w, router_b)
    max_n = max(len(i) for i in idxs)
    C = _capacity(max_n)

    nc = _get_nc(C)
    in_maps = _prep_core_inputs(x2d, idxs, cws, gate_w, up_w, down_w, C)
    res = run_bass_kernel_spmd(nc, in_maps, core_ids=list(range(N_EXPERTS)), trace=False)

    out = np.zeros((B * S, D_MODEL), np.float32)
    for e in range(N_EXPERTS):
        n = len(idxs[e])
        np.add.at(out, idxs[e], res.results[e]["y"].T[:n])
    return out.reshape(B, S, D_MODEL)


# revision 27
# speedup vs baseline: 1.0349x; 1.0349x over previous
"""MoE FFN (8 experts, top-2) on 8 TRN2 NeuronCores, expert-parallel.

Strategy:
  - Host: router (fp64 logits -> softmax -> top-2 -> renormalized combine
    weights), gather each expert's assigned tokens, pad to a common
    capacity C (SPMD: one program, per-core inputs).
  - Core e: full SwiGLU FFN for expert e over its C tokens in bf16
    (full PE rate, ~40 MB HBM traffic/rep), combine-weight scaling on
    device; outputs [C, 1024].
  - Host: scatter-add per-expert outputs back into [B, S, D].

Layouts (host-prepared, DMA-friendly):
  xT   [8, 128, C]       x[idx].T split along d into 8 k-tiles (bf16)
  gw/uw[32, 128, 8, 128] gate/up ^T tiled: [h_tile][d_sub][k][h] (bf16)
  dw   [32, 128, 1024]   down^T tiled:     [h_tile][h_sub][dout] (bf16)
  cwT  [128, ceil(C/128)] combine weights, partition-major (f32)

Device schedule (per rep):
  Phase 1: for each of 32 h-tiles, gate+up matmuls over ~416-token PSUM
    groups (8 accumulating k-matmuls each, same PSUM bank per chain),
    silu on Act engine, h=sg*u on DVE into a resident bf16 hbuf
    [128, 32, C].
  Phase 2: down-proj with TOKENS as the moving dim (512-wide): per
    dout-tile (8 of 128), a dw slab [128, 32, 128]; per token group a
    single-bank 32-matmul accumulation chain (consecutive matmuls share
    the PSUM bank so the PE hides stationary loads); elementwise
    combine-weight scale on DVE (cw replicated across partitions), DMA
    out in [dout, token] layout; host transposes (free).

C is a multiple of 32 (tail token group of C%512 keeps capacity tight).
"""
import sys, os
for p in ("/opt/trn_rl_repo", os.path.join(os.path.dirname(os.path.abspath(__file__)))):
    if p not in sys.path:
        sys.path.insert(0, p)
import numpy as np
import ml_dtypes

D_MODEL = 1024
D_INNER = 4096
N_EXPERTS = 8
TOP_K = 2
H_TILES = D_INNER // 128  # 32
K_TILES = D_MODEL // 128  # 8


def _capacity(max_n: int) -> int:
    return max(256, ((max_n + 31) // 32) * 32)


def _token_groups(C: int):
    """Near-equal gate/up token groups, each a multiple of 32, <= 512."""
    n = (C + 511) // 512
    t32 = C // 32
    base, rem = divmod(t32, n)
    out, g0 = [], 0
    for i in range(n):
        gsz = (base + (1 if i < rem else 0)) * 32
        out.append((g0, gsz))
        g0 += gsz
    return out


def _build_nc(C: int, reps: int = 1, unroll: int = 4):
    import concourse.bass as bass
    import concourse.mybir as mybir
    import concourse.tile as tile
    from concourse import bacc
    from contextlib import nullcontext

    f32 = mybir.dt.float32
    bf16 = mybir.dt.bfloat16
    Silu = mybir.ActivationFunctionType.Silu

    assert C % 32 == 0
    groups = _token_groups(C)

    nc = bacc.Bacc(None, target_bir_lowering=False)
    xT_d = nc.dram_tensor("xT", [K_TILES, 128, C], bf16, kind="ExternalInput")
    # gate+up packed: [h_tile][d_sub][2(g,u)][k][h]
    guw_d = nc.dram_tensor("guw", [H_TILES, 128, 2, K_TILES, 128], bf16,
                           kind="ExternalInput")
    # dw packed per dout-tile: [dout_tile][h_sub][h_tile][dout_sub]
    dw_d = nc.dram_tensor("dw", [D_MODEL // 128, 128, H_TILES, 128], bf16,
                          kind="ExternalInput")
    cw_d = nc.dram_tensor("cwR", [128, C], f32, kind="ExternalInput")
    y_d = nc.dram_tensor("y", [D_MODEL, C], f32, kind="ExternalOutput")

    with tile.TileContext(nc) as tc:
        with (
            tc.tile_pool(name="xt", bufs=1) as xt_pool,
            tc.tile_pool(name="wgt", bufs=3) as wgt_pool,
            tc.tile_pool(name="dwp", bufs=2) as dw_pool,
            tc.tile_pool(name="hb", bufs=1) as hb_pool,
            tc.tile_pool(name="sg", bufs=2) as sg_pool,
            tc.tile_pool(name="yo", bufs=3) as y_pool,
            tc.tile_pool(name="cw", bufs=1) as cw_pool,
            tc.tile_pool(name="ps1", bufs=3, space="PSUM") as ps1,
            tc.tile_pool(name="ps2", bufs=5, space="PSUM") as ps2,
        ):
            cw_sb = cw_pool.tile([128, C], f32)
            nc.sync.dma_start(cw_sb[:], cw_d[:])

            def emit_rep():
                # x streamed per token group into separate tiles so the first
                # gate matmul only waits on its own group's DMA.
                xts = []
                for gi, (g0, gsz) in enumerate(groups):
                    xg = xt_pool.tile([128, K_TILES, gsz], bf16, tag=f"xt{gi}",
                                      name="xg")
                    # DRAM [k, d, t] -> SBUF [d, k, t] (transpose on the DRAM
                    # side so the SBUF AP stays partition-major)
                    nc.sync.dma_start(xg[:], xT_d[:, :, g0:g0 + gsz].transpose([1, 0, 2]))
                    xts.append(xg)
                hbuf = hb_pool.tile([128, H_TILES, C], bf16, tag="hbuf")

                # ---- phase 1: gate/up + SwiGLU, weights streamed once
                for hi in range(H_TILES):
                    guw = wgt_pool.tile([128, 2, K_TILES, 128], bf16, tag="w")
                    nc.sync.dma_start(guw[:], guw_d[hi])
                    for gi, (g0, gsz) in enumerate(groups):
                        hs = slice(g0, g0 + gsz)
                        xg = xts[gi]
                        pg = ps1.tile([128, gsz], f32, tag="p1", name="pg",
                                      padded_shape=[128, 512])
                        for k in range(K_TILES):
                            nc.tensor.matmul(pg[:], guw[:, 0, k, :], xg[:, k, :],
                                             start=(k == 0), stop=(k == K_TILES - 1))
                        pu = ps1.tile([128, gsz], f32, tag="p1", name="pu",
                                      padded_shape=[128, 512])
                        for k in range(K_TILES):
                            nc.tensor.matmul(pu[:], guw[:, 1, k, :], xg[:, k, :],
                                             start=(k == 0), stop=(k == K_TILES - 1))
                        sg = sg_pool.tile([128, gsz], bf16, tag="sg", name="sg",
                                          padded_shape=[128, 512])
                        nc.scalar.activation(sg[:], pg[:], Silu)
                        nc.vector.tensor_mul(hbuf[:, hi, hs], sg[:], pu[:])

                # ---- phase 2: down-projection, tokens moving (512-wide),
                # same-bank 32-matmul accumulation chains per token group
                for dt in range(D_MODEL // 128):
                    dwt = dw_pool.tile([128, H_TILES, 128], bf16, tag="dw", name="dwt")
                    nc.sync.dma_start(dwt[:], dw_d[dt])
                    for (g0, gsz) in groups:
                        gs = slice(g0, g0 + gsz)
                        yp = ps2.tile([128, gsz], f32, tag="p2", name="yp",
                                      padded_shape=[128, 512])
                        for hi in range(H_TILES):
                            nc.tensor.matmul(
                                yp[:], dwt[:, hi, :], hbuf[:, hi, gs],
                                start=(hi == 0), stop=(hi == H_TILES - 1))
                        yt = y_pool.tile([128, gsz], f32, tag="yt", name="yt",
                                         padded_shape=[128, 512])
                        nc.vector.tensor_mul(yt[:], yp[:], cw_sb[:, gs])
                        nc.sync.dma_start(y_d[dt * 128:(dt + 1) * 128, gs], yt[:])

            # Unroll the rep loop (per-For_i-iteration overhead is ~30 ns/rep
            # x measured ~30 us at unroll=1); tail reps emitted inline.
            n_loop, tail = divmod(reps, unroll)
            if n_loop > 1:
                with tc.For_i(0, n_loop, 1):
                    for _u in range(unroll):
                        emit_rep()
                for _u in range(tail):
                    emit_rep()
            else:
                for _u in range(max(1, reps)):
                    emit_rep()
    nc.finalize()
    return nc


_NC_CACHE: dict = {}


def _get_nc(C: int):
    if C not in _NC_CACHE:
        _NC_CACHE[C] = _build_nc(C)
    return _NC_CACHE[C]


def _route(x2d: np.ndarray, router_w: np.ndarray, router_b: np.ndarray):
    """fp64 router: returns (idx_per_expert, cw_per_expert) lists."""
    logits = x2d.astype(np.float64) @ router_w.astype(np.float64).T + router_b.astype(np.float64)
    m = logits.max(axis=-1, keepdims=True)
    p = np.exp(logits - m)
    p /= p.sum(axis=-1, keepdims=True)
    # top-2 (jax.lax.top_k picks largest; softmax is monotonic in logits)
    i1 = np.argmax(p, axis=-1)
    p_masked = p.copy()
    p_masked[np.arange(p.shape[0]), i1] = -1.0
    i2 = np.argmax(p_masked, axis=-1)
    p1 = p[np.arange(p.shape[0]), i1]
    p2 = p[np.arange(p.shape[0]), i2]
    denom = p1 + p2
    w1 = p1 / denom
    w2 = p2 / denom
    idxs, cws = [], []
    for e in range(N_EXPERTS):
        sel1 = np.nonzero(i1 == e)[0]
        sel2 = np.nonzero(i2 == e)[0]
        idx = np.concatenate([sel1, sel2])
        cw = np.concatenate([w1[sel1], w2[sel2]])
        idxs.append(idx)
        cws.append(cw.astype(np.float32))
    return idxs, cws


def _prep_core_inputs(x2d, idxs, cws, gate_w, up_w, down_w, C):
    bf16 = ml_dtypes.bfloat16
    in_maps = []
    for e in range(N_EXPERTS):
        idx = idxs[e]
        n = len(idx)
        xe = np.zeros((C, D_MODEL), np.float32)
        xe[:n] = x2d[idx]
        xT = np.ascontiguousarray(xe.T).astype(bf16).reshape(K_TILES, 128, C)
        gw = gate_w[e].T.reshape(K_TILES, 128, H_TILES, 128).transpose(2, 1, 0, 3)
        uw = up_w[e].T.reshape(K_TILES, 128, H_TILES, 128).transpose(2, 1, 0, 3)
        # [h_tile][d_sub][2(g,u)][k][h]
        guw = np.ascontiguousarray(
            np.stack([gw, uw], axis=2)
        ).astype(bf16)
        # [dout_tile][h_sub][h_tile][dout_sub]
        dw = np.ascontiguousarray(
            down_w[e].T.reshape(H_TILES, 128, D_MODEL // 128, 128).transpose(2, 1, 0, 3)
        ).astype(bf16)
        cw = np.zeros((C,), np.float32)
        cw[:n] = cws[e]
        cwR = np.ascontiguousarray(np.broadcast_to(cw[None, :], (128, C)))
        in_maps.append({"xT": xT, "guw": guw, "dw": dw, "cwR": cwR})
    return in_maps


def kernel(x, router_w, router_b, gate_w, up_w, down_w):
    from concourse.bass_utils import run_bass_kernel_spmd

    x = np.asarray(x, dtype=np.float32)
    router_w = np.asarray(router_w, dtype=np.float32)
    router_b = np.asarray(router_b, dtype=np.float32)
    gate_w = np.asarray(gate_w, dtype=np.float32)
    up_w = np.asarray(up_w, dtype=np.float32)
    down_w = np.asarray(down_w, dtype=np.float32)

    B, S, D = x.shape
    x2d = x.reshape(B * S, D)
    idxs, cws = _route(x2d, router_w, router_b)
    max_n = max(len(i) for i in idxs)
    C = _capacity(max_n)

    nc = _get_nc(C)
    in_maps = _prep_core_inputs(x2d, idxs, cws, gate_w, up_w, down_w, C)
    res = run_bass_kernel_spmd(nc, in_maps, core_ids=list(range(N_EXPERTS)), trace=False)

    out = np.zeros((B * S, D_MODEL), np.float32)
    for e in range(N_EXPERTS):
        n = len(idxs[e])
        np.add.at(out, idxs[e], res.results[e]["y"].T[:n])
    return out.reshape(B, S, D_MODEL)
